# revision 21
# baseline (speedup 1.0000x reference)
"""Multi-head attention on 8 Trainium2 NeuronCores (tensor-parallel over heads).

B=4, S=2048, D=1024, H=16 heads of DK=64. Each core owns 2 heads (a
128-channel slice of the QKV projections). Per core, per batch b:
  xT   = transpose(x[b])           [d=128 x 8, S]  (DMA transpose, bf16,
                                   chunks alternate between 2 DMA queues)
  QT   = (Wq_c)^T x^T + bq_c       [128, S]        (channels on partitions)
  KT   = (Wk_c)^T x^T + bk_c       [128, S]
  V    = x Wv_c + bv_c             [S, 128] stored per-head with a ones col
  attention runs in 4 q-sections of 512 columns; per section, per k-chunk:
    sc  = [K_h0 Q_h0^T | K_h1 Q_h1^T]  [128, 1024] psum — the two matmuls
          have contraction 64 on partition halves 0-63 / 64-127, so they
          run CONCURRENTLY in different PE row groups (tile_position)
    ex  = exp(sc / 8)              bf16, ONE activation covers both heads
    av_h += V_h_aug^T ex_h         [65, 512] psum; rows 0-63 ctx^T, row 64
                                   the softmax denominator (ones column)
  ctxT = av[0:64] * recip(av[64])  fast-approx recip per head row,
                                   partition-broadcast on the gpsimd
                                   queue (no DRAM bounce)
  out[b] partial = ctx^T Wo_c      [S, D] bf16 staging -> DRAM (host
                                   sums partials in f64 and adds bo)

Matmul inputs are bf16; accumulation is fp32 in PSUM; softmax stats are
fp32. The emission is software-pipelined: each q-section's kc loop is
interleaved with the next batch's projections and the previous section's
normalize + output projection, so all engines stay fed.
"""

import numpy as np

B, S, D, H, DK = 4, 2048, 1024, 16, 64
NCORES = 8
CS = D // NCORES  # 128 channels (2 heads) per core
NSB = S // 128    # 16 s-blocks (also k-chunks)
NST = S // 512    # 4 s-tiles
NQP = S // 512    # 4 q-sections of 512
NDC = D // 128    # 8 d-chunks

TRACE = False
LAST_RESULTS = None
_CACHE = {}


def _interleave(main, fill, start_frac=0.0):
    """Spread fill groups evenly between main units (order preserved).
    ``fill`` is a list of unit-groups; a group's units are emitted
    consecutively (relative to other fill) so a group may pass psum
    tiles between its units without pool-rotation races."""
    out = []
    fi = 0
    n0 = int(len(main) * start_frac)
    for i, u in enumerate(main):
        out.append(u)
        if i < n0:
            continue
        want = (i - n0 + 1) * len(fill) // max(1, len(main) - n0)
        while fi < want:
            out.extend(fill[fi])
            fi += 1
    for g in fill[fi:]:
        out.extend(g)
    return out


def _build(repeat=1, bench_io=False):
    import concourse.bass as bass  # noqa: F401
    import concourse.mybir as mybir
    import concourse.tile as tile
    from concourse import bacc

    fp32 = mybir.dt.float32
    cdt = mybir.dt.bfloat16
    AF = mybir.ActivationFunctionType

    nc = bacc.Bacc(None, target_bir_lowering=False)
    if bench_io:
        x_d = nc.dram_tensor("xint", [B, S, D], cdt)
        out_d = nc.dram_tensor("outint", [B, S, D], cdt)
        xin_d = nc.declare_dram_parameter("xin", [128, 128], fp32, isOutput=False)
        xout_d = nc.declare_dram_parameter("xout", [128, 128], fp32, isOutput=True)
    else:
        x_d = nc.declare_dram_parameter("x", [B, S, D], cdt, isOutput=False)
        out_d = nc.declare_dram_parameter("out", [B, S, D], cdt, isOutput=True)
    wq_d = nc.declare_dram_parameter("wq", [D, CS], cdt, isOutput=False)
    wk_d = nc.declare_dram_parameter("wk", [D, CS], cdt, isOutput=False)
    wv_d = nc.declare_dram_parameter("wv", [D, CS], cdt, isOutput=False)
    wo_d = nc.declare_dram_parameter("wo", [CS, D], cdt, isOutput=False)
    bq_d = nc.declare_dram_parameter("bq", [CS], fp32, isOutput=False)
    bk_d = nc.declare_dram_parameter("bk", [CS], fp32, isOutput=False)
    bv_d = nc.declare_dram_parameter("bv", [CS], fp32, isOutput=False)

    with tile.TileContext(nc) as tc:
        with (
            tc.tile_pool(name="consts", bufs=1) as consts,
            tc.tile_pool(name="xt", bufs=2) as xt_pool,
            tc.tile_pool(name="qk", bufs=2) as qk_pool,
            tc.tile_pool(name="vp", bufs=2) as v_pool,
            tc.tile_pool(name="exp", bufs=4) as exp_pool,
            tc.tile_pool(name="ctx", bufs=2) as ctx_pool,
            tc.tile_pool(name="avs", bufs=4) as avs_pool,
            tc.tile_pool(name="sums", bufs=4) as sums_pool,
            tc.tile_pool(name="rcp", bufs=4) as rcp_pool,
            tc.tile_pool(name="rb", bufs=8) as rb_pool,
            tc.tile_pool(name="outp", bufs=4) as out_pool,
            tc.tile_pool(name="drp", bufs=8, space="DRAM") as dram_pool,
            tc.tile_pool(name="pssc", bufs=2, space="PSUM") as ps_sc,
            tc.tile_pool(name="psav", bufs=1, space="PSUM") as ps_av,
            tc.tile_pool(name="ps512", bufs=2, space="PSUM") as ps512,
        ):
            # ---- constants (tiles now, loads deferred until after the
            # first x-transpose DMAs are queued) ----
            wq_t = consts.tile([128, NDC, CS], cdt, tag="wq")
            wk_t = consts.tile([128, NDC, CS], cdt, tag="wk")
            wv_t = consts.tile([128, NDC, CS], cdt, tag="wv")
            wo_t = consts.tile([128, D], cdt, tag="wo")
            bq_t = consts.tile([128, 1], fp32, tag="bq")
            bk_t = consts.tile([128, 1], fp32, tag="bk")
            bv_b = consts.tile([128, CS], fp32, tag="bvb")

            def load_consts():
                nc.sync.dma_start(
                    wq_t[:], wq_d[:].rearrange("(c p) m -> p c m", p=128)
                )
                nc.sync.dma_start(
                    wk_t[:], wk_d[:].rearrange("(c p) m -> p c m", p=128)
                )
                nc.sync.dma_start(
                    wv_t[:], wv_d[:].rearrange("(c p) m -> p c m", p=128)
                )
                nc.sync.dma_start(wo_t[:], wo_d[:])
                nc.sync.dma_start(bq_t[:], bq_d[:].rearrange("(p o) -> p o", o=1))
                nc.sync.dma_start(bk_t[:], bk_d[:].rearrange("(p o) -> p o", o=1))
                nc.sync.dma_start(
                    bv_b[:],
                    bv_d[:].rearrange("(o f) -> o f", o=1).partition_broadcast(128),
                )
                if bench_io:
                    tio = consts.tile([128, 128], fp32, tag="tio")
                    nc.sync.dma_start(tio[:], xin_d[:])
                    nc.sync.dma_start(xout_d[:], tio[:])

            state = {}

            def A_units(bi, b):
                """x transpose + QKV projections for batch index bi.
                Returns (xdma_units, proj_units); proj units are fine-
                grained (a few hundred ns of PE each) so they can slot
                into the ACT-bound attention sections as filler."""
                xT = xt_pool.tile([128, NDC, S], cdt, tag="xT")
                QT = qk_pool.tile([128, S], cdt, tag="QT")
                KT = qk_pool.tile([128, S], cdt, tag="KT")
                v0 = v_pool.tile([128, NSB, 65], cdt, tag="v0")
                v1 = v_pool.tile([128, NSB, 65], cdt, tag="v1")
                state[bi] = dict(xT=xT, QT=QT, KT=KT, v0=v0, v1=v1)

                xdmas = []
                xr = x_d[b].rearrange("M (c p) -> M c p", p=128)
                for cch in range(NDC):
                    # all transposes on the sync HWDGE queue: the scalar
                    # queue variant produced small nondeterministic
                    # corruption on HW (sim-clean), so it is off-limits
                    xdmas.append(
                        lambda cch=cch: nc.sync.dma_start(
                            xT[:, cch, :], xr[:, cch], transpose=True
                        )
                    )

                qk_groups = {}
                # Q / K projections: per s-tile of 512, two 4-chunk matmul
                # halves (~850ns of PE each) forming one atomic group —
                # the halves share a psum tile, and only main (attention)
                # units may interleave between them, which never allocate
                # from ps512.
                for st in range(NST):
                    for wt, bt, dst in ((wq_t, bq_t, QT), (wk_t, bk_t, KT)):
                        pbox = []

                        def u_qk0(st=st, wt=wt, pbox=pbox):
                            sl = slice(st * 512, (st + 1) * 512)
                            p = ps512.tile([128, 512], fp32, tag="mm512")
                            pbox.append(p)
                            for cch in range(4):
                                nc.tensor.matmul(
                                    p[:], wt[:, cch, :], xT[:, cch, sl],
                                    start=(cch == 0), stop=False,
                                )

                        def u_qk1(st=st, wt=wt, bt=bt, dst=dst, pbox=pbox):
                            sl = slice(st * 512, (st + 1) * 512)
                            p = pbox[0]
                            for cch in range(4, NDC):
                                nc.tensor.matmul(
                                    p[:], wt[:, cch, :], xT[:, cch, sl],
                                    start=False, stop=(cch == NDC - 1),
                                )
                            nc.vector.tensor_scalar_add(dst[:, sl], p[:], bt[:])

                        qk_groups[("q" if dst is QT else "k", st)] = [u_qk0, u_qk1]

                def u_ones():
                    nc.gpsimd.memset(v0[:, :, 64:65], 1.0)
                    nc.gpsimd.memset(v1[:, :, 64:65], 1.0)
                v_groups = []
                for sb in range(NSB):
                    def u_v(sb=sb):
                        pv = ps512.tile([128, 128], fp32, tag="mm512")
                        for cch in range(NDC):
                            nc.tensor.matmul(
                                pv[:], xT[:, cch, sb * 128 : (sb + 1) * 128],
                                wv_t[:, cch, :],
                                start=(cch == 0), stop=(cch == NDC - 1),
                            )
                        nc.vector.tensor_add(
                            v0[:, sb, 0:64], pv[:, 0:64], bv_b[:, 0:64]
                        )
                        nc.vector.tensor_add(
                            v1[:, sb, 0:64], pv[:, 64:128], bv_b[:, 64:128]
                        )
                    v_groups.append([u_v])
                # emission order: K (whole S — needed by every q-section's
                # kc loop), Q st0 (first section), first half of V, ones,
                # then the rest. The section-0 interleave keeps each V
                # block at least one kc-slot ahead of the av matmul that
                # consumes it.
                head = (
                    [qk_groups[("k", st)] for st in range(NST)]
                    + [qk_groups[("q", 0)]]
                    + v_groups[:8]
                    + [[u_ones]]
                )
                rest = v_groups[8:] + [qk_groups[("q", st)] for st in (1, 2, 3)]
                return xdmas, head + rest

            def B_section(bi, qp):
                """One 512-column q-section: 16 kc slots (scores pair +
                exp + av pair, software-pipelined with one-kc lag), then
                the psum drain + batched reciprocal + normalize chain.
                Returns the unit list for the section."""
                st_ = state[bi]
                QT, KT, v0, v1 = st_["QT"], st_["KT"], st_["v0"], st_["v1"]
                ctx = st_["ctx"]
                qsl = slice(qp * 512, (qp + 1) * 512)
                units = []
                loc = {}

                def u_alloc():
                    loc["av0"] = ps_av.tile([65, 512], fp32, tag="av0", name="av0")
                    loc["av1"] = ps_av.tile([65, 512], fp32, tag="av1", name="av1")
                units.append(u_alloc)

                exq = {}

                def u_sc(kc):
                    ksl = slice(kc * 128, (kc + 1) * 128)
                    sc = ps_sc.tile([128, 1024], fp32, tag="sc")
                    # the two heads' score matmuls use partition halves
                    # 0-63 / 64-127 => different PE row groups => they
                    # execute concurrently
                    nc.tensor.matmul(
                        sc[:, 0:512], KT[0:64, ksl], QT[0:64, qsl],
                        start=True, stop=True,
                    )
                    nc.tensor.matmul(
                        sc[:, 512:1024], KT[64:128, ksl], QT[64:128, qsl],
                        start=True, stop=True,
                    )
                    ex = exp_pool.tile([128, 1024], cdt, tag="ex")
                    nc.scalar.activation(ex[:], sc[:], AF.Exp, scale=0.125)
                    exq[kc] = ex

                def u_av(kc):
                    ex = exq.pop(kc)
                    nc.tensor.matmul(
                        loc["av0"][:], v0[:, kc, :], ex[:, 0:512],
                        start=(kc == 0), stop=(kc == NSB - 1),
                        skip_group_check=True,
                    )
                    nc.tensor.matmul(
                        loc["av1"][:], v1[:, kc, :], ex[:, 512:1024],
                        start=(kc == 0), stop=(kc == NSB - 1),
                        skip_group_check=True,
                    )

                for kc in range(NSB):
                    def u_kc(kc=kc):
                        u_sc(kc)
                        if kc > 0:
                            u_av(kc - 1)
                        if kc == NSB - 1:
                            u_av(kc)
                    units.append(u_kc)

                def u_drain():
                    # free the two av psum banks fast (copies only), then
                    # pull the two denominator rows into one [2,512] tile
                    # via SBUF->SBUF DMA (cross-partition move)
                    av0, av1 = loc["av0"], loc["av1"]
                    s2 = sums_pool.tile([2, 512], fp32, tag="s2")
                    a0 = avs_pool.tile([65, 512], fp32, tag="avs")
                    a1 = avs_pool.tile([65, 512], fp32, tag="avs")
                    nc.vector.tensor_copy(a0[:], av0[:])
                    nc.vector.tensor_copy(a1[:], av1[:])
                    nc.sync.dma_start(s2[0:1, :], a0[64:65, :])
                    nc.sync.dma_start(s2[1:2, :], a1[64:65, :])
                    loc["a0"], loc["a1"], loc["s2"] = a0, a1, s2
                units.append(u_drain)

                def u_recip():
                    rc = rcp_pool.tile([2, 512], fp32, tag="rc")
                    nc.vector.reciprocal_approx_fast(rc[:], loc["s2"][:])
                    dr = dram_pool.tile([2, 512], fp32, tag="dr")
                    nc.sync.dma_start(dr[:], rc[:])
                    rb0 = rb_pool.tile([64, 512], fp32, tag="rb")
                    rb1 = rb_pool.tile([64, 512], fp32, tag="rb")
                    nc.sync.dma_start(rb0[:], dr[0:1, :].partition_broadcast(64))
                    nc.sync.dma_start(rb1[:], dr[1:2, :].partition_broadcast(64))
                    loc["rb0"], loc["rb1"] = rb0, rb1
                units.append(u_recip)

                def u_norm():
                    nc.vector.tensor_mul(
                        ctx[0:64, qsl], loc["a0"][0:64, :], loc["rb0"][:]
                    )
                    nc.vector.tensor_mul(
                        ctx[64:128, qsl], loc["a1"][0:64, :], loc["rb1"][:]
                    )
                units.append(u_norm)
                return units

            def C_units(bi, b, qp):
                """Output projection for one q-section (4 s-blocks of 128).
                DMAs the result straight from PSUM to DRAM. Returns unit
                groups."""
                ctx = state[bi]["ctx"]
                groups = []
                for sb in range(qp * 4, qp * 4 + 4):
                    for half in range(2):
                        def u_o(sb=sb, half=half, ctx=ctx):
                            lsl = slice(sb * 128, (sb + 1) * 128)
                            osl = slice(half * 512, (half + 1) * 512)
                            po = ps512.tile([128, 512], fp32, tag="mm512")
                            nc.tensor.matmul(
                                po[:], ctx[:, lsl], wo_t[:, osl],
                                start=True, stop=True,
                            )
                            ot = out_pool.tile([128, 512], cdt, tag="ot")
                            nc.scalar.copy(ot[:], po[:])
                            nc.sync.dma_start(
                                out_d[b, sb * 128 : (sb + 1) * 128, osl], ot[:]
                            )
                        groups.append([u_o])
                return groups

            bs = [bb for _ in range(repeat) for bb in range(B)]

            # ---- prologue: batch 0 x-transposes, consts, K/Q0/V0-7 ----
            xdmas0, a0 = A_units(0, bs[0])
            load_consts()
            for u in xdmas0:
                u()
            for g in a0[:14]:
                for u in g:
                    u()
            a0_rest = a0[14:]
            state[0]["ctx"] = ctx_pool.tile([128, S], cdt, tag="ctx", name="ctx")

            # ---- main software-pipelined loop ----
            # section (bi, qp) is interleaved with:
            #   - next batch's x-dmas (issued at qp==0 start)
            #   - next batch's projection groups (spread over qp 1..3;
            #     qp1 uses start_frac so the PE never waits on an
            #     x-transpose that hasn't landed yet)
            #   - C groups of section qp-1 (same batch) or qp3 of bi-1
            c_carry = []  # C unit-groups of previous section
            for bi, b in enumerate(bs):
                if bi + 1 < len(bs):
                    xdmas_n, a_next = A_units(bi + 1, bs[bi + 1])
                    state[bi + 1]["ctx"] = ctx_pool.tile(
                        [128, S], cdt, tag="ctx", name="ctx"
                    )
                else:
                    xdmas_n, a_next = [], []
                for qp in range(NQP):
                    sec = B_section(bi, qp)
                    sf = 0.0
                    if qp == 0:
                        for u in xdmas_n[:4]:
                            u()
                        fill = a0_rest + c_carry
                        a0_rest = []
                    elif qp == 1:
                        for u in xdmas_n[4:]:
                            u()
                        fill = c_carry + a_next[: len(a_next) // 3]
                        sf = 0.5
                    else:
                        n = len(a_next)
                        lo = n * (qp - 1) // (NQP - 1)
                        hi = n * qp // (NQP - 1)
                        fill = c_carry + a_next[lo:hi]
                    for u in _interleave(sec, fill, start_frac=sf):
                        u()
                    c_carry = C_units(bi, b, qp)
                del state[bi]
            for g in c_carry:
                for u in g:
                    u()

    nc.compile()
    return nc


def _get_nc(repeat=1, bench_io=False):
    key = f"nc{repeat}_{bench_io}"
    if key not in _CACHE:
        _CACHE[key] = _build(repeat, bench_io)
    return _CACHE[key]


def kernel(**inputs):
    global LAST_RESULTS
    import ml_dtypes
    from concourse.bass_utils import run_bass_kernel_spmd

    cdt = ml_dtypes.bfloat16
    x = np.ascontiguousarray(np.asarray(inputs["x"], dtype=np.float32).astype(cdt))
    Wq = np.asarray(inputs["Wq"], dtype=np.float32).astype(cdt)
    Wk = np.asarray(inputs["Wk"], dtype=np.float32).astype(cdt)
    Wv = np.asarray(inputs["Wv"], dtype=np.float32).astype(cdt)
    Wo = np.asarray(inputs["Wo"], dtype=np.float32).astype(cdt)
    bq = np.asarray(inputs["bq"], dtype=np.float32)
    bk = np.asarray(inputs["bk"], dtype=np.float32)
    bv = np.asarray(inputs["bv"], dtype=np.float32)
    bo = np.asarray(inputs["bo"], dtype=np.float32)

    nc = _get_nc()
    in_maps = []
    for c in range(NCORES):
        cs = slice(CS * c, CS * (c + 1))
        in_maps.append(
            {
                "x": x,
                "wq": np.ascontiguousarray(Wq[:, cs]),
                "wk": np.ascontiguousarray(Wk[:, cs]),
                "wv": np.ascontiguousarray(Wv[:, cs]),
                "wo": np.ascontiguousarray(Wo[cs, :]),
                "bq": np.ascontiguousarray(bq[cs]),
                "bk": np.ascontiguousarray(bk[cs]),
                "bv": np.ascontiguousarray(bv[cs]),
            }
        )
    res = run_bass_kernel_spmd(
        nc, in_maps, core_ids=list(range(NCORES)), trace=TRACE
    )
    LAST_RESULTS = res
    acc = np.zeros((B, S, D), dtype=np.float64)
    for c in range(NCORES):
        acc += res.results[c]["out"]
    acc += bo
    return acc.astype(np.float32)


# revision 22
# speedup vs baseline: 1.0291x; 1.0291x over previous
"""Multi-head attention on 8 Trainium2 NeuronCores (tensor-parallel over heads).

B=4, S=2048, D=1024, H=16 heads of DK=64. Each core owns 2 heads (a
128-channel slice of the QKV projections). Per core, per batch b:
  xT   = transpose(x[b])           [d=128 x 8, S]  (DMA transpose, bf16,
                                   chunks alternate between 2 DMA queues)
  QT   = (Wq_c)^T x^T + bq_c       [128, S]        (channels on partitions)
  KT   = (Wk_c)^T x^T + bk_c       [128, S]
  V    = x Wv_c + bv_c             [S, 128] stored per-head with a ones col
  attention runs in 4 q-sections of 512 columns; per section, per k-chunk:
    sc  = [K_h0 Q_h0^T | K_h1 Q_h1^T]  [128, 1024] psum — the two matmuls
          have contraction 64 on partition halves 0-63 / 64-127, so they
          run CONCURRENTLY in different PE row groups (tile_position)
    ex  = exp(sc / 8)              bf16, ONE activation covers both heads
    av_h += V_h_aug^T ex_h         [65, 512] psum; rows 0-63 ctx^T, row 64
                                   the softmax denominator (ones column)
  ctxT = av[0:64] * recip(av[64])  recip batched [2, 512] per section,
                                   broadcast via DRAM bounce
  out[b] partial = ctx^T Wo_c      [S, D] fp32, DMA'd DRAM-ward straight
                                   from PSUM (host sums partials + bo)

Matmul inputs are bf16; accumulation is fp32 in PSUM; softmax stats and
the output are fp32. The emission is software-pipelined: each q-section's
kc loop (ACT-bound) is interleaved with the next batch's projections, the
previous section's normalize + output projection, so all engines stay fed.
"""

import numpy as np

B, S, D, H, DK = 4, 2048, 1024, 16, 64
NCORES = 8
CS = D // NCORES  # 128 channels (2 heads) per core
NSB = S // 128    # 16 s-blocks (also k-chunks)
NST = S // 512    # 4 s-tiles
NQP = S // 512    # 4 q-sections of 512
NDC = D // 128    # 8 d-chunks

TRACE = False
LAST_RESULTS = None
_CACHE = {}


def _interleave(main, fill, start_frac=0.0):
    """Spread fill groups evenly between main units (order preserved).
    ``fill`` is a list of unit-groups; a group's units are emitted
    consecutively (relative to other fill) so a group may pass psum
    tiles between its units without pool-rotation races."""
    out = []
    fi = 0
    n0 = int(len(main) * start_frac)
    for i, u in enumerate(main):
        out.append(u)
        if i < n0:
            continue
        want = (i - n0 + 1) * len(fill) // max(1, len(main) - n0)
        while fi < want:
            out.extend(fill[fi])
            fi += 1
    for g in fill[fi:]:
        out.extend(g)
    return out


def _build(repeat=1, bench_io=False):
    import concourse.bass as bass  # noqa: F401
    import concourse.mybir as mybir
    import concourse.tile as tile
    from concourse import bacc

    fp32 = mybir.dt.float32
    cdt = mybir.dt.bfloat16
    AF = mybir.ActivationFunctionType

    nc = bacc.Bacc(None, target_bir_lowering=False)
    if bench_io:
        x_d = nc.dram_tensor("xint", [B, S, D], cdt)
        out_d = nc.dram_tensor("outint", [B, S, D], cdt)
        xin_d = nc.declare_dram_parameter("xin", [128, 128], fp32, isOutput=False)
        xout_d = nc.declare_dram_parameter("xout", [128, 128], fp32, isOutput=True)
    else:
        x_d = nc.declare_dram_parameter("x", [B, S, D], cdt, isOutput=False)
        out_d = nc.declare_dram_parameter("out", [B, S, D], cdt, isOutput=True)
    wq_d = nc.declare_dram_parameter("wq", [D, CS], cdt, isOutput=False)
    wk_d = nc.declare_dram_parameter("wk", [D, CS], cdt, isOutput=False)
    wv_d = nc.declare_dram_parameter("wv", [D, CS], cdt, isOutput=False)
    wo_d = nc.declare_dram_parameter("wo", [CS, D], cdt, isOutput=False)
    bq_d = nc.declare_dram_parameter("bq", [CS], fp32, isOutput=False)
    bk_d = nc.declare_dram_parameter("bk", [CS], fp32, isOutput=False)
    bv_d = nc.declare_dram_parameter("bv", [CS], fp32, isOutput=False)

    with tile.TileContext(nc) as tc:
        with (
            tc.tile_pool(name="consts", bufs=1) as consts,
            tc.tile_pool(name="xt", bufs=2) as xt_pool,
            tc.tile_pool(name="qk", bufs=2) as qk_pool,
            tc.tile_pool(name="vp", bufs=2) as v_pool,
            tc.tile_pool(name="exp", bufs=4) as exp_pool,
            tc.tile_pool(name="ctx", bufs=2) as ctx_pool,
            tc.tile_pool(name="avs", bufs=4) as avs_pool,
            tc.tile_pool(name="sums", bufs=4) as sums_pool,
            tc.tile_pool(name="rcp", bufs=4) as rcp_pool,
            tc.tile_pool(name="rb", bufs=8) as rb_pool,
            tc.tile_pool(name="outp", bufs=4) as out_pool,
            tc.tile_pool(name="drp", bufs=8, space="DRAM") as dram_pool,
            tc.tile_pool(name="pssc", bufs=2, space="PSUM") as ps_sc,
            tc.tile_pool(name="psav", bufs=1, space="PSUM") as ps_av,
            tc.tile_pool(name="ps512", bufs=2, space="PSUM") as ps512,
        ):
            # ---- constants (tiles now, loads deferred until after the
            # first x-transpose DMAs are queued) ----
            wq_t = consts.tile([128, NDC, CS], cdt, tag="wq")
            wk_t = consts.tile([128, NDC, CS], cdt, tag="wk")
            wv_t = consts.tile([128, NDC, CS], cdt, tag="wv")
            wo_t = consts.tile([128, D], cdt, tag="wo")
            bq_t = consts.tile([128, 1], fp32, tag="bq")
            bk_t = consts.tile([128, 1], fp32, tag="bk")
            bv_b = consts.tile([128, CS], fp32, tag="bvb")

            def load_consts():
                nc.sync.dma_start(
                    wq_t[:], wq_d[:].rearrange("(c p) m -> p c m", p=128)
                )
                nc.sync.dma_start(
                    wk_t[:], wk_d[:].rearrange("(c p) m -> p c m", p=128)
                )
                nc.sync.dma_start(
                    wv_t[:], wv_d[:].rearrange("(c p) m -> p c m", p=128)
                )
                nc.sync.dma_start(wo_t[:], wo_d[:])
                nc.sync.dma_start(bq_t[:], bq_d[:].rearrange("(p o) -> p o", o=1))
                nc.sync.dma_start(bk_t[:], bk_d[:].rearrange("(p o) -> p o", o=1))
                nc.sync.dma_start(
                    bv_b[:],
                    bv_d[:].rearrange("(o f) -> o f", o=1).partition_broadcast(128),
                )
                if bench_io:
                    tio = consts.tile([128, 128], fp32, tag="tio")
                    nc.sync.dma_start(tio[:], xin_d[:])
                    nc.sync.dma_start(xout_d[:], tio[:])

            state = {}

            def A_units(bi, b):
                """x transpose + QKV projections for batch index bi.
                Returns (xdma_units, proj_units); proj units are fine-
                grained (a few hundred ns of PE each) so they can slot
                into the ACT-bound attention sections as filler."""
                xT = xt_pool.tile([128, NDC, S], cdt, tag="xT")
                QT = qk_pool.tile([128, S], cdt, tag="QT")
                KT = qk_pool.tile([128, S], cdt, tag="KT")
                v0 = v_pool.tile([128, NSB, 65], cdt, tag="v0")
                v1 = v_pool.tile([128, NSB, 65], cdt, tag="v1")
                state[bi] = dict(xT=xT, QT=QT, KT=KT, v0=v0, v1=v1)

                xdmas = []
                xr = x_d[b].rearrange("M (c p) -> M c p", p=128)
                for cch in range(NDC):
                    # all transposes on the sync HWDGE queue: the scalar
                    # queue variant produced small nondeterministic
                    # corruption on HW (sim-clean), so it is off-limits
                    xdmas.append(
                        lambda cch=cch: nc.sync.dma_start(
                            xT[:, cch, :], xr[:, cch], transpose=True
                        )
                    )

                qk_groups = {}
                # Q / K projections: per s-tile of 512, two 4-chunk matmul
                # halves (~850ns of PE each) forming one atomic group —
                # the halves share a psum tile, and only main (attention)
                # units may interleave between them, which never allocate
                # from ps512.
                for st in range(NST):
                    for wt, bt, dst in ((wq_t, bq_t, QT), (wk_t, bk_t, KT)):
                        pbox = []

                        def u_qk0(st=st, wt=wt, pbox=pbox):
                            sl = slice(st * 512, (st + 1) * 512)
                            p = ps512.tile([128, 512], fp32, tag="mm512")
                            pbox.append(p)
                            for cch in range(4):
                                nc.tensor.matmul(
                                    p[:], wt[:, cch, :], xT[:, cch, sl],
                                    start=(cch == 0), stop=False,
                                )

                        def u_qk1(st=st, wt=wt, bt=bt, dst=dst, pbox=pbox):
                            sl = slice(st * 512, (st + 1) * 512)
                            p = pbox[0]
                            for cch in range(4, NDC):
                                nc.tensor.matmul(
                                    p[:], wt[:, cch, :], xT[:, cch, sl],
                                    start=False, stop=(cch == NDC - 1),
                                )
                            nc.vector.tensor_scalar_add(dst[:, sl], p[:], bt[:])

                        qk_groups[("q" if dst is QT else "k", st)] = [u_qk0, u_qk1]

                def u_ones():
                    nc.gpsimd.memset(v0[:, :, 64:65], 1.0)
                    nc.gpsimd.memset(v1[:, :, 64:65], 1.0)
                v_groups = []
                for sb in range(NSB):
                    def u_v(sb=sb):
                        pv = ps512.tile([128, 128], fp32, tag="mm512")
                        for cch in range(NDC):
                            nc.tensor.matmul(
                                pv[:], xT[:, cch, sb * 128 : (sb + 1) * 128],
                                wv_t[:, cch, :],
                                start=(cch == 0), stop=(cch == NDC - 1),
                            )
                        nc.vector.tensor_add(
                            v0[:, sb, 0:64], pv[:, 0:64], bv_b[:, 0:64]
                        )
                        nc.vector.tensor_add(
                            v1[:, sb, 0:64], pv[:, 64:128], bv_b[:, 64:128]
                        )
                    v_groups.append([u_v])
                # emission order: K (whole S — needed by every q-section's
                # kc loop), Q st0 (first section), first half of V, ones,
                # then the rest. The section-0 interleave keeps each V
                # block at least one kc-slot ahead of the av matmul that
                # consumes it.
                head = (
                    [qk_groups[("k", st)] for st in range(NST)]
                    + [qk_groups[("q", 0)]]
                    + v_groups[:8]
                    + [[u_ones]]
                )
                rest = v_groups[8:] + [qk_groups[("q", st)] for st in (1, 2, 3)]
                return xdmas, head + rest

            def B_section(bi, qp):
                """One 512-column q-section: 16 kc slots (scores pair +
                exp + av pair, software-pipelined with one-kc lag), then
                the psum drain + batched reciprocal + normalize chain.
                Returns the unit list for the section."""
                st_ = state[bi]
                QT, KT, v0, v1 = st_["QT"], st_["KT"], st_["v0"], st_["v1"]
                ctx = st_["ctx"]
                qsl = slice(qp * 512, (qp + 1) * 512)
                units = []
                loc = {}

                def u_alloc():
                    loc["av0"] = ps_av.tile([65, 512], fp32, tag="av0", name="av0")
                    loc["av1"] = ps_av.tile([65, 512], fp32, tag="av1", name="av1")
                units.append(u_alloc)

                exq = {}

                def u_sc(kc):
                    ksl = slice(kc * 128, (kc + 1) * 128)
                    sc = ps_sc.tile([128, 1024], fp32, tag="sc")
                    # the two heads' score matmuls use partition halves
                    # 0-63 / 64-127 => different PE row groups => they
                    # execute concurrently
                    nc.tensor.matmul(
                        sc[:, 0:512], KT[0:64, ksl], QT[0:64, qsl],
                        start=True, stop=True,
                    )
                    nc.tensor.matmul(
                        sc[:, 512:1024], KT[64:128, ksl], QT[64:128, qsl],
                        start=True, stop=True,
                    )
                    ex = exp_pool.tile([128, 1024], cdt, tag="ex")
                    nc.scalar.activation(ex[:], sc[:], AF.Exp, scale=0.125)
                    exq[kc] = ex

                def u_av(kc):
                    ex = exq.pop(kc)
                    nc.tensor.matmul(
                        loc["av0"][:], v0[:, kc, :], ex[:, 0:512],
                        start=(kc == 0), stop=(kc == NSB - 1),
                        skip_group_check=True,
                    )
                    nc.tensor.matmul(
                        loc["av1"][:], v1[:, kc, :], ex[:, 512:1024],
                        start=(kc == 0), stop=(kc == NSB - 1),
                        skip_group_check=True,
                    )

                for kc in range(NSB):
                    def u_kc(kc=kc):
                        u_sc(kc)
                        if kc > 0:
                            u_av(kc - 1)
                        if kc == NSB - 1:
                            u_av(kc)
                    units.append(u_kc)

                def u_drain():
                    # free the two av psum banks fast (copies only), then
                    # pull the two denominator rows into one [2,512] tile
                    # via SBUF->SBUF DMA (cross-partition move)
                    av0, av1 = loc["av0"], loc["av1"]
                    s2 = sums_pool.tile([2, 512], fp32, tag="s2")
                    a0 = avs_pool.tile([65, 512], fp32, tag="avs")
                    a1 = avs_pool.tile([65, 512], fp32, tag="avs")
                    nc.vector.tensor_copy(a0[:], av0[:])
                    nc.vector.tensor_copy(a1[:], av1[:])
                    nc.sync.dma_start(s2[0:1, :], a0[64:65, :])
                    nc.sync.dma_start(s2[1:2, :], a1[64:65, :])
                    loc["a0"], loc["a1"], loc["s2"] = a0, a1, s2
                units.append(u_drain)

                def u_recip():
                    rc = rcp_pool.tile([2, 512], fp32, tag="rc")
                    nc.vector.reciprocal_approx_fast(rc[:], loc["s2"][:])
                    dr = dram_pool.tile([2, 512], fp32, tag="dr")
                    nc.sync.dma_start(dr[:], rc[:])
                    rb0 = rb_pool.tile([64, 512], fp32, tag="rb")
                    rb1 = rb_pool.tile([64, 512], fp32, tag="rb")
                    nc.sync.dma_start(rb0[:], dr[0:1, :].partition_broadcast(64))
                    nc.sync.dma_start(rb1[:], dr[1:2, :].partition_broadcast(64))
                    loc["rb0"], loc["rb1"] = rb0, rb1
                units.append(u_recip)

                def u_norm():
                    nc.vector.tensor_mul(
                        ctx[0:64, qsl], loc["a0"][0:64, :], loc["rb0"][:]
                    )
                    nc.vector.tensor_mul(
                        ctx[64:128, qsl], loc["a1"][0:64, :], loc["rb1"][:]
                    )
                units.append(u_norm)
                return units

            def C_units(bi, b, qp):
                """Output projection for one q-section (4 s-blocks of 128).
                DMAs the result straight from PSUM to DRAM. Returns unit
                groups."""
                ctx = state[bi]["ctx"]
                groups = []
                for sb in range(qp * 4, qp * 4 + 4):
                    for half in range(2):
                        def u_o(sb=sb, half=half, ctx=ctx):
                            lsl = slice(sb * 128, (sb + 1) * 128)
                            osl = slice(half * 512, (half + 1) * 512)
                            po = ps512.tile([128, 512], fp32, tag="mm512")
                            nc.tensor.matmul(
                                po[:], ctx[:, lsl], wo_t[:, osl],
                                start=True, stop=True,
                            )
                            ot = out_pool.tile([128, 512], cdt, tag="ot")
                            nc.vector.tensor_copy(ot[:], po[:])
                            nc.sync.dma_start(
                                out_d[b, sb * 128 : (sb + 1) * 128, osl], ot[:]
                            )
                        groups.append([u_o])
                return groups

            bs = [bb for _ in range(repeat) for bb in range(B)]

            # ---- prologue: batch 0 x-transposes, consts, K/Q0/V0-7 ----
            xdmas0, a0 = A_units(0, bs[0])
            load_consts()
            for u in xdmas0:
                u()
            for g in a0[:14]:
                for u in g:
                    u()
            a0_rest = a0[14:]
            state[0]["ctx"] = ctx_pool.tile([128, S], cdt, tag="ctx", name="ctx")

            # ---- main software-pipelined loop ----
            # section (bi, qp) is interleaved with:
            #   - next batch's x-dmas (issued at qp==0 start)
            #   - next batch's projection groups (spread over qp 1..3;
            #     qp1 uses start_frac so the PE never waits on an
            #     x-transpose that hasn't landed yet)
            #   - C groups of section qp-1 (same batch) or qp3 of bi-1
            c_carry = []  # C unit-groups of previous section
            for bi, b in enumerate(bs):
                if bi + 1 < len(bs):
                    xdmas_n, a_next = A_units(bi + 1, bs[bi + 1])
                    state[bi + 1]["ctx"] = ctx_pool.tile(
                        [128, S], cdt, tag="ctx", name="ctx"
                    )
                else:
                    xdmas_n, a_next = [], []
                for qp in range(NQP):
                    sec = B_section(bi, qp)
                    sf = 0.0
                    if qp == 0:
                        for u in xdmas_n:
                            u()
                        fill = a0_rest + c_carry
                        a0_rest = []
                    else:
                        n = len(a_next)
                        lo = n * (qp - 1) // (NQP - 1)
                        hi = n * qp // (NQP - 1)
                        fill = c_carry + a_next[lo:hi]
                        if qp == 1:
                            sf = 0.5
                    for u in _interleave(sec, fill, start_frac=sf):
                        u()
                    c_carry = C_units(bi, b, qp)
                del state[bi]
            for g in c_carry:
                for u in g:
                    u()

    nc.compile()
    return nc


def _get_nc(repeat=1, bench_io=False):
    key = f"nc{repeat}_{bench_io}"
    if key not in _CACHE:
        _CACHE[key] = _build(repeat, bench_io)
    return _CACHE[key]


def kernel(**inputs):
    global LAST_RESULTS
    import ml_dtypes
    from concourse.bass_utils import run_bass_kernel_spmd

    cdt = ml_dtypes.bfloat16
    x = np.ascontiguousarray(np.asarray(inputs["x"], dtype=np.float32).astype(cdt))
    Wq = np.asarray(inputs["Wq"], dtype=np.float32).astype(cdt)
    Wk = np.asarray(inputs["Wk"], dtype=np.float32).astype(cdt)
    Wv = np.asarray(inputs["Wv"], dtype=np.float32).astype(cdt)
    Wo = np.asarray(inputs["Wo"], dtype=np.float32).astype(cdt)
    bq = np.asarray(inputs["bq"], dtype=np.float32)
    bk = np.asarray(inputs["bk"], dtype=np.float32)
    bv = np.asarray(inputs["bv"], dtype=np.float32)
    bo = np.asarray(inputs["bo"], dtype=np.float32)

    nc = _get_nc()
    in_maps = []
    for c in range(NCORES):
        cs = slice(CS * c, CS * (c + 1))
        in_maps.append(
            {
                "x": x,
                "wq": np.ascontiguousarray(Wq[:, cs]),
                "wk": np.ascontiguousarray(Wk[:, cs]),
                "wv": np.ascontiguousarray(Wv[:, cs]),
                "wo": np.ascontiguousarray(Wo[cs, :]),
                "bq": np.ascontiguousarray(bq[cs]),
                "bk": np.ascontiguousarray(bk[cs]),
                "bv": np.ascontiguousarray(bv[cs]),
            }
        )
    res = run_bass_kernel_spmd(
        nc, in_maps, core_ids=list(range(NCORES)), trace=TRACE
    )
    LAST_RESULTS = res
    acc = np.zeros((B, S, D), dtype=np.float64)
    for c in range(NCORES):
        acc += res.results[c]["out"]
    acc += bo
    return acc.astype(np.float32)
